# revision 9
# baseline (speedup 1.0000x reference)
"""Trainium2 Bass kernel for a 2-layer GAT network (nn_GATNet).

Sharding: nodes permuted host-side (degree-sorted, snake-dealt across 8
cores). Table layout is GROUP-major: 5 groups x 8 tiles; within a group,
8 cores x 8 tiles x 128 rows, so each group's AllGather writes one
contiguous table slice. Row 0 / row NROWS-1 are pad rows (written locally
by every core, never AllGathered) whose a_src = -1e4 makes exp == 0 for
pad slots. Per conv: each core builds its table tiles (xh | a_src | junk
as 512B f16 rows via one matmul per tile; same matmul yields a_dst kept
in SBUF), fires a chunked AllGather per finished group (overlapping the
next group's build and the previous phase), then processes its
destination-bucketed edges: dst node (t,p) owns partition p of tile t,
incoming edges in slot columns fetched by dma_gather (int16 indices =>
two overlapping <=32768-row windows A/B; 1024 idx max per gather; 4 SWDGE
queues round-robin). Softmax skips max-subtraction (alphas are O(1)).
Slot accumulation is an identity-matmul into PSUM, 3 slots per matmul,
combined on DVE. Epilogue: normalize, layernorm, prelu, PE-transpose back
to [feat, node]; conv2's table build is interleaved into conv1's edge
loop so its AllGathers hide behind conv1 edge processing.
"""

import os
import numpy as np
import ml_dtypes

import concourse.bacc as bacc
import concourse.tile as tile
import concourse.bass as bass
import concourse.mybir as mybir
from concourse.bass_utils import run_bass_kernel_spmd
from concourse.masks import make_identity

F16 = np.float16

N, E = 40000, 640000
EMB, HID, H, TXT = 128, 128, 4, 384
C = HID // H
NCORES = 8
P = 128
RTILES = 40                      # real node tiles per core
GT = int(os.environ.get("GAT_GT", "40"))  # tiles per collective group
NG = RTILES // GT                # groups (collectives per layer)
CHUNK = NCORES * GT * P          # table rows per group (8192)
SHARD = RTILES * P               # 5120 real rows per core
NROWS = 2 + NCORES * SHARD      # 40962: row 0 = padA, last = padB
ROWB = 256                       # f16 elements per table row (512 B)
WINA = 32767                     # window A = rows [0, 32767)
WINB_BASE = NROWS - 32768        # 8194; window B = rows [8194, 40962)
PADA_ROW = 0
PADB_ROW = NROWS - 1             # 40961; idx in B = 32767
REAL = N // NCORES               # 5000 real nodes per core
LN_EPS = 1e-5
ASR_PAD = -1.0e4                 # pad-row a_src => exp(leaky(...)) == 0

_cache = {}


# ---------------------------------------------------------------- host side

def _pack_idx(flat):
    """Flat int list -> [128, n/16] int16 wrapped layout for dma_gather."""
    n = len(flat)
    assert n % 16 == 0
    a = np.asarray(flat)
    assert a.min() >= 0 and a.max() <= 32767, (a.min(), a.max())
    t = a.astype(np.int16).reshape(n // 16, 16).T      # [16, n/16]
    return np.ascontiguousarray(np.tile(t, (8, 1)))    # [128, n/16]


def _perm_from_order(order):
    """order (rank -> orig node) => (row_of, nodes_of_core)."""
    r = np.arange(N)
    blk, pos = r // NCORES, r % NCORES
    core_of_rank = np.where(blk % 2 == 0, pos, NCORES - 1 - pos)
    node_core = np.empty(N, np.int64)
    node_slot = np.empty(N, np.int64)
    node_core[order] = core_of_rank
    node_slot[order] = blk                      # local slot 0..4999
    t = node_slot // P
    p = node_slot % P
    row_of = 1 + (t // GT) * CHUNK + node_core * (GT * P) + (t % GT) * P + p
    nodes_of_core = [order[core_of_rank == c] for c in range(NCORES)]
    return row_of, nodes_of_core


def _preprocess(edge_index):
    src = np.concatenate([edge_index[0].astype(np.int64), np.arange(N)])
    dst = np.concatenate([edge_index[1].astype(np.int64), np.arange(N)])
    indeg = np.bincount(dst, minlength=N)

    # pass 1: degree-sorted; pass 2-3: refine with forced-A counts so tiles
    # (consecutive 1024-rank blocks) are homogeneous in (deg, fa)
    order = np.argsort(-indeg, kind="stable")
    row_of, nodes_of_core = _perm_from_order(order)
    for _ in range(2):
        srow = row_of[src]
        fa_cnt = np.bincount(dst[srow < WINB_BASE], minlength=N)
        order = np.lexsort((-fa_cnt, -indeg))
        row_of, nodes_of_core = _perm_from_order(order)

    e_src_row = row_of[src]
    e_dst_row = row_of[dst]
    eorder = np.argsort(e_dst_row, kind="stable")
    s_src = e_src_row[eorder]
    s_dst = e_dst_row[eorder]
    bounds = np.searchsorted(s_dst, np.arange(NROWS + 1))

    # tile (kA, kB): minimal feasible given per-node forced-A/forced-B counts
    # and degrees: kA >= max fa, kB >= max fb, kA + kB >= max deg.
    node_fa = {}
    node_fb = {}
    node_fl = {}
    faM = np.zeros((NCORES, RTILES), np.int64)
    fbM = np.zeros((NCORES, RTILES), np.int64)
    dgM = np.zeros((NCORES, RTILES), np.int64)
    for c in range(NCORES):
        for t in range(RTILES):
            base = 1 + (t // GT) * CHUNK + c * (GT * P) + (t % GT) * P
            for p in range(P):
                grow = base + p
                lo, hi = bounds[grow], bounds[grow + 1]
                if lo == hi:
                    continue
                srcs = s_src[lo:hi]
                fa = srcs[srcs < WINB_BASE]
                fb = srcs[srcs >= WINA]
                fl = srcs[(srcs >= WINB_BASE) & (srcs < WINA)]
                node_fa[grow] = fa
                node_fb[grow] = fb
                node_fl[grow] = fl
                faM[c, t] = max(faM[c, t], len(fa))
                fbM[c, t] = max(fbM[c, t], len(fb))
                dgM[c, t] = max(dgM[c, t], hi - lo)

    skA = np.maximum(faM.max(axis=0), 1)
    skB = np.maximum(fbM.max(axis=0), 1)
    need = np.maximum(dgM.max(axis=0) - (skA + skB), 0)
    skA = skA + (need + 1) // 2
    skB = skB + need // 2
    sched = tuple((int(skA[t]), int(skB[t])) for t in range(RTILES))

    node_A = {}
    node_B = {}
    tile_of_row = np.zeros(NROWS, np.int64)
    for c in range(NCORES):
        for t in range(RTILES):
            base = 1 + (t // GT) * CHUNK + c * (GT * P) + (t % GT) * P
            tile_of_row[base:base + P] = t
    for grow, fa in node_fa.items():
        t = tile_of_row[grow]
        fb = node_fb[grow]
        fl = node_fl[grow]
        deg = len(fa) + len(fb) + len(fl)
        a_d = int(np.clip(deg - int(skB[t]), len(fa), len(fa) + len(fl)))
        node_A[grow] = np.concatenate([fa, fl[: a_d - len(fa)]])
        node_B[grow] = np.concatenate([fb, fl[a_d - len(fa):]]) - WINB_BASE

    idxa_cols, idxb_cols = [], []
    for c in range(NCORES):
        fa_all, fb_all = [], []
        for t in range(RTILES):
            ka, kb = sched[t]
            arrA = np.full((P, ka), PADA_ROW, np.int64)
            arrB = np.full((P, kb), PADB_ROW - WINB_BASE, np.int64)
            base = 1 + (t // GT) * CHUNK + c * (GT * P) + (t % GT) * P
            for p in range(P):
                grow = base + p
                la = node_A.get(grow)
                if la is not None and len(la):
                    arrA[p, : len(la)] = la
                lb = node_B.get(grow)
                if lb is not None and len(lb):
                    arrB[p, : len(lb)] = lb
            fa_all.append(arrA.T.reshape(-1))
            fb_all.append(arrB.T.reshape(-1))
        idxa_cols.append(_pack_idx(np.concatenate(fa_all)))
        idxb_cols.append(_pack_idx(np.concatenate(fb_all)))

    return {
        "sched": sched,
        "nodes_of_core": nodes_of_core,
        "idxa": idxa_cols,
        "idxb": idxb_cols,
    }


def _wext(conv_w, att_src, att_dst):
    """[128, 136] rhs: 0:128 conv_w.T | 128:132 a_src w | 132:136 a_dst w."""
    w = np.zeros((HID, HID + 2 * H), np.float32)
    w[:, :HID] = conv_w.T
    wr = conv_w.reshape(H, C, HID)
    w[:, HID:HID + H] = np.einsum("hc,hcf->fh", att_src, wr)
    w[:, HID + H:] = np.einsum("hc,hcf->fh", att_dst, wr)
    return w


def _bc(vec):
    return np.ascontiguousarray(np.tile(np.asarray(vec, np.float32)[None, :], (P, 1)))


# ---------------------------------------------------------------- bass build

def _build(sched, flags):
    STAGE = int(os.environ.get("GAT_STAGE", "9"))
    g_is1, b_is0, cb_is0, pa_scalar = flags
    nc = bacc.Bacc("TRN2", target_bir_lowering=False, debug=False,
                   enable_asserts=True, num_devices=NCORES, num_swdge_queues=4)
    dt = mybir.dt
    f32, f16, i16, i32 = dt.float32, dt.float16, dt.int16, dt.int32

    nA = 8 * sum(k for k, _ in sched)
    nB = 8 * sum(k for _, k in sched)

    def din(name, shape, dtype):
        return nc.dram_tensor(name, shape, dtype, kind="ExternalInput").ap()

    xT = din("xT", [EMB, SHARD], f16)
    txtT = din("txtT", [TXT, SHARD], f16)
    numT = din("numT", [1, SHARD], f16)
    idxa = din("idxa", [P, nA], i16)
    idxb = din("idxb", [P, nB], i16)
    npwT = din("npwT", [EMB, HID], f16)
    tpwT = din("tpwT", [TXT, HID], f16)
    numwT = din("numwT", [1, HID], f16)
    bias0 = din("bias0", [P, 1], f32)
    prelu0a = din("prelu0a", [P, 1], f32)
    w1ext = din("w1ext", [HID, HID + 2 * H], f16)
    w2ext = din("w2ext", [HID, HID + 2 * H], f16)
    padrow = din("padrow", [1, ROWB], f16)
    cb1 = din("cb1", [P, HID], f32)
    g1 = din("g1", [P, HID], f32)
    bln1 = din("bln1", [P, HID], f32)
    pa1 = din("pa1", [P, HID], f32)
    cb2 = din("cb2", [P, HID], f32)
    g2 = din("g2", [P, HID], f32)
    bln2 = din("bln2", [P, HID], f32)
    pa2 = din("pa2", [P, HID], f32)
    outw = din("outw", [P, HID], f32)
    outb = din("outb", [P, 1], f32)

    out = nc.dram_tensor("out", [SHARD, 1], f32, kind="ExternalOutput").ap()

    # per-layer, per-group collective inputs (local) and full tables (shared)
    cc_g = [[nc.dram_tensor(f"cc{li}_g{g}", [GT * P, ROWB], f16)
             for g in range(NG)] for li in (0, 1)]
    table = [nc.dram_tensor(f"table{i}", [NROWS, ROWB], f16, addr_space="Shared")
             for i in (1, 2)]
    table_ap = [tt.ap() for tt in table]

    def bc_ap(ap, t_count, at=1):
        new = list(map(list, ap.ap))
        new.insert(at, [0, t_count])
        return bass.AP(tensor=ap.tensor, offset=ap.offset, ap=new)

    def app_ap(ap, count):
        new = list(map(list, ap.ap)) + [[0, count]]
        return bass.AP(tensor=ap.tensor, offset=ap.offset, ap=new)

    qctr = [0]

    def next_q():
        qctr[0] += 1
        return qctr[0] % 4

    with tile.TileContext(nc) as tc:
        consts = tc.alloc_tile_pool(name="consts", bufs=1)
        persist = tc.alloc_tile_pool(name="persist", bufs=1)
        io = tc.alloc_tile_pool(name="io", bufs=2)
        work = tc.alloc_tile_pool(name="work", bufs=2)
        ep = tc.alloc_tile_pool(name="ep", bufs=2)
        psA = tc.alloc_tile_pool(name="psA", bufs=2, space="PSUM")
        psB = tc.alloc_tile_pool(name="psB", bufs=2, space="PSUM")
        psC = tc.alloc_tile_pool(name="psC", bufs=2, space="PSUM")
        psT = tc.alloc_tile_pool(name="psT", bufs=2, space="PSUM")

        _ld_n = [0]

        def ld(ap_in, shape, dtype, pool=consts):
            _ld_n[0] += 1
            nm = f"const{_ld_n[0]}"
            t = pool.tile(shape, dtype, name=nm, tag=nm)
            nc.sync.dma_start(out=t[:], in_=ap_in)
            return t

        sb_idxa = ld(idxa, [P, nA], i16)
        sb_idxb = ld(idxb, [P, nB], i16)
        sb_npwT = ld(npwT, [EMB, HID], f16)
        sb_tpwT = [ld(ch, [P, HID], f16) for ch in
                   (tpwT[0:P, :], tpwT[P:2 * P, :], tpwT[2 * P:3 * P, :])]
        sb_numwT = ld(numwT, [1, HID], f16)
        sb_bias0 = ld(bias0, [P, 1], f32)
        sb_pr0a = ld(prelu0a, [P, 1], f32)
        sb_wext = [ld(w1ext, [HID, HID + 2 * H], f16),
                   ld(w2ext, [HID, HID + 2 * H], f16)]
        sb_cb = [ld(cb1, [P, HID], f32), ld(cb2, [P, HID], f32)]
        sb_g = [ld(g1, [P, HID], f32), ld(g2, [P, HID], f32)]
        sb_bln = [ld(bln1, [P, HID], f32), ld(bln2, [P, HID], f32)]
        sb_pa = [ld(pa1, [P, HID], f32), ld(pa2, [P, HID], f32)]
        sb_outw = ld(outw, [P, HID], f32)
        sb_outb = ld(outb, [P, 1], f32)

        ident16 = consts.tile([P, P], f16)
        make_identity(nc, ident16[:])
        identf = consts.tile([P, P], f32)
        make_identity(nc, identf[:])
        eps_t = consts.tile([P, 1], f32)
        nc.vector.memset(eps_t[:], LN_EPS)

        hT = [persist.tile([P, SHARD], f16, tag=f"hT{i}", name=f"hT{i}")
              for i in range(1)]
        # conv1 output kept as per-tile tiles so conv2's table build can
        # start before the whole conv1 edge phase finishes
        h1t = [persist.tile([P, P], f16, tag=f"h1t{t}", name=f"h1t{t}")
               for t in range(RTILES)]
        adst_all = persist.tile([P, 2, RTILES, H], f16)

        # pad rows: every core writes its own table copies locally
        for li in range(2):
            nc.sync.dma_start(out=table_ap[li][PADA_ROW:PADA_ROW + 1, :],
                              in_=padrow[0:1, :])
            nc.sync.dma_start(out=table_ap[li][PADB_ROW:PADB_ROW + 1, :],
                              in_=padrow[0:1, :])

        def build_tile(li, t, lhsT_t):
            """Emit table-row build for tile t of layer li."""
            ps = psB.tile([P, HID + 2 * H], f32, tag="tb")
            nc.tensor.matmul(ps[:], lhsT=lhsT_t,
                             rhs=sb_wext[li][:], start=True, stop=True)
            trow = work.tile([P, ROWB], f16, tag="trow")
            nc.scalar.copy(out=trow[:, 0:HID + H], in_=ps[:, 0:HID + H])
            g = t // GT
            r0 = (t % GT) * P
            nc.sync.dma_start(out=cc_g[li][g].ap()[r0:r0 + P, :], in_=trow[:])
            nc.vector.tensor_copy(out=adst_all[:, li, t, :],
                                  in_=ps[:, HID + H:HID + 2 * H])

        def fire_group(li, g):
            """AllGather group g of layer li into the shared table."""
            row0 = 1 + g * CHUNK
            nc.gpsimd.collective_compute(
                "AllGather", mybir.AluOpType.bypass,
                replica_groups=[list(range(NCORES))],
                ins=[cc_g[li][g].ap().opt()],
                outs=[table_ap[li][row0:row0 + CHUNK, :].opt()],
            )

        # ---- stage 1: h0T = prelu0(proj(x, txt, num) + bias0), transposed;
        # conv1 table build + group collectives interleaved per 512-col chunk
        col = 0
        built = 0
        while col < SHARD:
            cw = min(512, SHARD - col)
            sl = slice(col, col + cw)
            x_t = io.tile([P, cw], f16, tag="x")
            nc.sync.dma_start(out=x_t[:], in_=xT[:, sl])
            tx_t = [io.tile([P, cw], f16, tag=f"tx{k}", name=f"tx{k}")
                    for k in range(3)]
            for k in range(3):
                nc.sync.dma_start(out=tx_t[k][:], in_=txtT[k * P:(k + 1) * P, sl])
            nm_t = io.tile([1, cw], f16, tag="nm")
            nc.sync.dma_start(out=nm_t[:], in_=numT[0:1, sl])

            ps = psA.tile([P, cw], f32, tag="ps1")
            nc.tensor.matmul(ps[:], lhsT=sb_npwT[:], rhs=x_t[:],
                             start=True, stop=False)
            for k in range(3):
                nc.tensor.matmul(ps[:], lhsT=sb_tpwT[k][:], rhs=tx_t[k][:],
                                 start=False, stop=False)
            nc.tensor.matmul(ps[:], lhsT=sb_numwT[:], rhs=nm_t[:],
                             start=False, stop=True)
            nc.scalar.activation(out=hT[0][:, sl], in_=ps[:],
                                 func=mybir.ActivationFunctionType.Prelu,
                                 bias=sb_bias0[:], alpha=sb_pr0a[:])
            col += cw
            # table build for completed tiles
            done = min(col // P, RTILES)
            while built < done:
                t = built
                build_tile(0, t, hT[0][:, t * P:(t + 1) * P])
                built += 1
                if built % GT == 0:
                    fire_group(0, built // GT - 1)

        # ---- conv layers
        nconv = 0 if STAGE <= 1 else (2 if STAGE >= 9 else 1)
        for li in range(nconv):
            if STAGE <= 2:
                continue
            oa = ob = 0
            for t in range(RTILES):
                ka, kb = sched[t]
                T = ka + kb
                T3 = 3 * ((T + 2) // 3)
                G = work.tile([P, T, ROWB], f16, tag="G", bufs=2)
                for g0 in range(0, ka, 8):
                    n = min(8, ka - g0)
                    nc.gpsimd.dma_gather(
                        G[:, g0:g0 + n, :], table_ap[li][0:WINA, :],
                        sb_idxa[:, oa + g0 * 8:oa + (g0 + n) * 8],
                        n * P, n * P, ROWB, queue_num=next_q())
                for g0 in range(0, kb, 8):
                    n = min(8, kb - g0)
                    nc.gpsimd.dma_gather(
                        G[:, ka + g0:ka + g0 + n, :],
                        table_ap[li][WINB_BASE:NROWS, :],
                        sb_idxb[:, ob + g0 * 8:ob + (g0 + n) * 8],
                        n * P, n * P, ROWB, queue_num=next_q())
                oa += ka * 8
                ob += kb * 8
                if STAGE == 3:
                    continue

                alph = work.tile([P, T, H], f16, tag="alph")
                nc.vector.tensor_tensor(out=alph[:],
                                        in0=G[:, :, HID:HID + H],
                                        in1=bc_ap(adst_all[:, li, t, :], T),
                                        op=mybir.AluOpType.add)
                # leaky relu on DVE: max(a, 0.2a)
                nc.vector.scalar_tensor_tensor(out=alph[:], in0=alph[:],
                                               scalar=0.2, in1=alph[:],
                                               op0=mybir.AluOpType.mult,
                                               op1=mybir.AluOpType.max)
                RHS = work.tile([P, T3, HID + H], f16, tag="RHS")
                if T3 > T:
                    nc.vector.memset(RHS[:, T:T3, :], 0)
                nc.scalar.activation(out=RHS[:, 0:T, HID:HID + H], in_=alph[:],
                                     func=mybir.ActivationFunctionType.Exp)
                ex_b = RHS[:, 0:T, HID:HID + H]
                nc.vector.tensor_tensor(
                    out=RHS[:, 0:T, 0:HID].rearrange("p t (h c) -> p t h c", h=H),
                    in0=G[:, :, 0:HID].rearrange("p t (h c) -> p t h c", h=H),
                    in1=app_ap(ex_b, C), op=mybir.AluOpType.mult)

                if STAGE == 36:
                    continue
                ps2 = psC.tile([P, 3, HID + H], f32, tag="cv")
                ng = T3 // 3
                for g in range(ng):
                    nc.tensor.matmul(ps2[:], lhsT=ident16[:],
                                     rhs=RHS[:, 3 * g:3 * g + 3, :],
                                     start=(g == 0), stop=(g == ng - 1))
                nm2 = ep.tile([P, HID + H], f32, tag="nm2")
                ps2v = ps2[:]
                ps2_sw = bass.AP(
                    tensor=ps2v.tensor, offset=ps2v.offset,
                    ap=[list(ps2v.ap[0]), [1, HID + H], [HID + H, 3]])
                nc.vector.reduce_sum(out=nm2[:], in_=ps2_sw,
                                     axis=mybir.AxisListType.X)

                den = ep.tile([P, H], f32, tag="den")
                if t == RTILES - 1:
                    # +eps so empty dst rows (no edges, tail of the last
                    # tile) yield 0/eps = 0, not NaN
                    nc.vector.tensor_scalar_add(out=nm2[:, HID:HID + H],
                                                in0=nm2[:, HID:HID + H],
                                                scalar1=1e-30)
                nc.vector.reciprocal(out=den[:], in_=nm2[:, HID:HID + H])
                hb = ep.tile([P, HID], f32, tag="hb")
                nc.vector.tensor_tensor(
                    out=hb[:].rearrange("p (h c) -> p h c", h=H),
                    in0=nm2[:, 0:HID].rearrange("p (h c) -> p h c", h=H),
                    in1=app_ap(den[:], C), op=mybir.AluOpType.mult)
                if not cb_is0:
                    nc.vector.tensor_tensor(out=hb[:], in0=hb[:], in1=sb_cb[li][:],
                                            op=mybir.AluOpType.add)
                stats = ep.tile([P, nc.vector.BN_STATS_DIM], f32, tag="st")
                nc.vector.bn_stats(out=stats[:], in_=hb[:])
                mv = ep.tile([P, nc.vector.BN_AGGR_DIM], f32, tag="mv")
                nc.vector.bn_aggr(out=mv[:], in_=stats[:])
                sq = ep.tile([P, 1], f32, tag="sq")
                nc.scalar.activation(out=sq[:], in_=mv[:, 1:2],
                                     func=mybir.ActivationFunctionType.Sqrt,
                                     bias=eps_t[:])
                nc.vector.reciprocal(out=sq[:], in_=sq[:])
                if g_is1 and b_is0 and pa_scalar is not None:
                    # fused (hb - mean) * rstd then prelu, on the scalar engine
                    nbias = ep.tile([P, 1], f32, tag="nb")
                    nc.vector.scalar_tensor_tensor(out=nbias[:], in0=mv[:, 0:1],
                                                   scalar=-1.0, in1=sq[:],
                                                   op0=mybir.AluOpType.mult,
                                                   op1=mybir.AluOpType.mult)
                    hb2 = ep.tile([P, HID], f32, tag="hb2")
                    nc.scalar.activation(out=hb2[:], in_=hb[:],
                                         func=mybir.ActivationFunctionType.Prelu,
                                         bias=nbias[:], scale=sq[:],
                                         alpha=float(pa_scalar))
                    hb = hb2
                else:
                    # hb = (hb - mean) * rstd
                    nc.vector.tensor_scalar(out=hb[:], in0=hb[:], scalar1=mv[:, 0:1],
                                            scalar2=sq[:, 0:1],
                                            op0=mybir.AluOpType.subtract,
                                            op1=mybir.AluOpType.mult)
                    if not g_is1:
                        nc.vector.tensor_tensor(out=hb[:], in0=hb[:], in1=sb_g[li][:],
                                                op=mybir.AluOpType.mult)
                    if not b_is0:
                        nc.vector.tensor_tensor(out=hb[:], in0=hb[:], in1=sb_bln[li][:],
                                                op=mybir.AluOpType.add)
                    # prelu
                    t2 = ep.tile([P, HID], f32, tag="t2")
                    if pa_scalar is not None:
                        nc.vector.tensor_scalar(out=t2[:], in0=hb[:], scalar1=0.0,
                                                scalar2=float(pa_scalar),
                                                op0=mybir.AluOpType.min,
                                                op1=mybir.AluOpType.mult)
                    else:
                        nc.vector.tensor_scalar(out=t2[:], in0=hb[:], scalar1=0.0,
                                                scalar2=None, op0=mybir.AluOpType.min)
                        nc.vector.tensor_tensor(out=t2[:], in0=t2[:], in1=sb_pa[li][:],
                                                op=mybir.AluOpType.mult)
                    nc.vector.scalar_tensor_tensor(out=hb[:], in0=hb[:], scalar=0.0,
                                                   in1=t2[:], op0=mybir.AluOpType.max,
                                                   op1=mybir.AluOpType.add)

                if li == 0:
                    pst = psT.tile([P, P], f32, tag="tr")
                    nc.tensor.transpose(out=pst[:], in_=hb[:], identity=identf[:])
                    nc.vector.tensor_copy(out=h1t[t][:], in_=pst[:])
                    # conv2 table build rides the conv1 edge loop so its
                    # group collectives overlap conv1 edge processing
                    build_tile(1, t, h1t[t][:])
                    if (t + 1) % GT == 0:
                        fire_group(1, (t + 1) // GT - 1)
                else:
                    om = ep.tile([P, HID], f32, tag="om")
                    nc.vector.tensor_tensor(out=om[:], in0=hb[:], in1=sb_outw[:],
                                            op=mybir.AluOpType.mult)
                    ov = ep.tile([P, 1], f32, tag="ov")
                    nc.vector.reduce_sum(out=ov[:], in_=om[:],
                                         axis=mybir.AxisListType.X)
                    nc.vector.tensor_scalar_add(out=ov[:], in0=ov[:],
                                                scalar1=sb_outb[:, 0:1])
                    nc.sync.dma_start(out=out[t * P:(t + 1) * P, :], in_=ov[:])

        for p in (psT, psC, psB, psA, ep, work, io, persist, consts):
            p.release()

    nc.compile()
    return nc


# ---------------------------------------------------------------- entry point

def kernel(x, num_x, num_mask, txt_x, txt_mask, edge_index,
           num_proj_w, num_proj_b, txt_proj_w, txt_proj_b,
           node_proj_w, node_proj_b, prelu0_a,
           conv1_w, att_src1, att_dst1, bias1, norm1_g, norm1_b, prelu1_a,
           conv2_w, att_src2, att_dst2, bias2, norm2_g, norm2_b, prelu2_a,
           out_w, out_b, _trace=False):
    x = np.asarray(x, np.float32)
    edge_index = np.asarray(edge_index)

    g_is1 = bool(np.all(norm1_g == 1) and np.all(norm2_g == 1))
    b_is0 = bool(np.all(norm1_b == 0) and np.all(norm2_b == 0))
    cb_is0 = bool(np.all(np.asarray(bias1) == 0) and np.all(np.asarray(bias2) == 0))
    pa1a = np.asarray(prelu1_a, np.float32)
    pa2a = np.asarray(prelu2_a, np.float32)
    pa_scalar = float(pa1a[0]) if (np.all(pa1a == pa1a[0])
                                   and np.all(pa2a == pa1a[0])) else None
    flags = (g_is1, b_is0, cb_is0, pa_scalar)

    pre_key = (hash(edge_index.tobytes()), flags)
    if pre_key in _cache:
        pre, nc = _cache[pre_key]
    else:
        pre = _preprocess(edge_index)
        nc = _build(pre["sched"], flags)
        _cache[pre_key] = (pre, nc)

    numv = (np.asarray(num_x, np.float32)[:, 0] * np.asarray(num_mask, np.float32))
    txtv = np.asarray(txt_x, np.float32) * np.asarray(txt_mask, np.float32)[:, None]
    bias0 = (np.asarray(num_proj_b) + np.asarray(txt_proj_b)
             + np.asarray(node_proj_b)).astype(np.float32)

    padrow = np.zeros((1, ROWB), np.float32)
    padrow[0, HID:HID + H] = ASR_PAD

    shared = {
        "npwT": np.ascontiguousarray(np.asarray(node_proj_w, np.float32).T).astype(F16),
        "tpwT": np.ascontiguousarray(np.asarray(txt_proj_w, np.float32).T).astype(F16),
        "numwT": np.ascontiguousarray(np.asarray(num_proj_w, np.float32).T).astype(F16),
        "bias0": bias0[:, None],
        "prelu0a": np.asarray(prelu0_a, np.float32)[:, None],
        "w1ext": _wext(np.asarray(conv1_w, np.float32),
                       np.asarray(att_src1, np.float32),
                       np.asarray(att_dst1, np.float32)).astype(F16),
        "w2ext": _wext(np.asarray(conv2_w, np.float32),
                       np.asarray(att_src2, np.float32),
                       np.asarray(att_dst2, np.float32)).astype(F16),
        "padrow": padrow.astype(F16),
        "cb1": _bc(bias1), "g1": _bc(norm1_g), "bln1": _bc(norm1_b), "pa1": _bc(prelu1_a),
        "cb2": _bc(bias2), "g2": _bc(norm2_g), "bln2": _bc(norm2_b), "pa2": _bc(prelu2_a),
        "outw": _bc(np.asarray(out_w, np.float32)[0]),
        "outb": np.full((P, 1), np.asarray(out_b, np.float32)[0], np.float32),
    }

    in_maps = []
    for c in range(NCORES):
        nodes = pre["nodes_of_core"][c]
        xTa = np.zeros((EMB, SHARD), np.float32)
        xTa[:, :REAL] = x[nodes].T
        txtTa = np.zeros((TXT, SHARD), np.float32)
        txtTa[:, :REAL] = txtv[nodes].T
        numTa = np.zeros((1, SHARD), np.float32)
        numTa[0, :REAL] = numv[nodes]
        m = dict(shared)
        m["xT"] = xTa.astype(F16)
        m["txtT"] = txtTa.astype(F16)
        m["numT"] = numTa.astype(F16)
        m["idxa"] = pre["idxa"][c]
        m["idxb"] = pre["idxb"][c]
        in_maps.append(m)

    res = run_bass_kernel_spmd(nc, in_maps, core_ids=list(range(NCORES)),
                               trace=_trace)
    out_full = np.zeros(N, np.float32)
    for c in range(NCORES):
        out_full[pre["nodes_of_core"][c]] = res.results[c]["out"][:REAL, 0]
    if _trace:
        kernel._last_exec_ns = res.exec_time_ns
        kernel._last_trace = res.instructions_and_trace
    return out_full


# revision 12
# speedup vs baseline: 1.1783x; 1.1783x over previous
"""Trainium2 Bass kernel for a 2-layer GAT network (nn_GATNet).

Sharding: nodes permuted host-side (degree-sorted, snake-dealt across 8
cores). Table layout is GROUP-major: 5 groups x 8 tiles; within a group,
8 cores x 8 tiles x 128 rows, so each group's AllGather writes one
contiguous table slice. Row 0 / row NROWS-1 are pad rows (written locally
by every core, never AllGathered) whose a_src = -1e4 makes exp == 0 for
pad slots. Per conv: each core builds its table tiles (xh | a_src | junk
as 512B f16 rows via one matmul per tile; same matmul yields a_dst kept
in SBUF), fires a chunked AllGather per finished group (overlapping the
next group's build and the previous phase), then processes its
destination-bucketed edges: dst node (t,p) owns partition p of tile t,
incoming edges in slot columns fetched by dma_gather (int16 indices =>
two overlapping <=32768-row windows A/B; 1024 idx max per gather; 4 SWDGE
queues round-robin). Softmax skips max-subtraction (alphas are O(1)).
Slot accumulation is an identity-matmul into PSUM, 3 slots per matmul,
combined on DVE. Epilogue: normalize, layernorm, prelu, PE-transpose back
to [feat, node]; conv2's table build is interleaved into conv1's edge
loop so its AllGathers hide behind conv1 edge processing.
"""

import os
import numpy as np
import ml_dtypes

import concourse.bacc as bacc
import concourse.tile as tile
import concourse.bass as bass
import concourse.mybir as mybir
from concourse.bass_utils import run_bass_kernel_spmd
from concourse.masks import make_identity

F16 = np.float16

N, E = 40000, 640000
EMB, HID, H, TXT = 128, 128, 4, 384
C = HID // H
NCORES = 8
P = 128
RTILES = 40                      # real node tiles per core
GT = int(os.environ.get("GAT_GT", "40"))  # tiles per collective group
NG = RTILES // GT                # groups (collectives per layer)
CHUNK = NCORES * GT * P          # table rows per group (8192)
SHARD = RTILES * P               # 5120 real rows per core
NROWS = 2 + NCORES * SHARD      # 40962: row 0 = padA, last = padB
ROWB = 256                       # f16 elements per table row (512 B)
WINA = 32767                     # window A = rows [0, 32767)
WINB_BASE = NROWS - 32768        # 8194; window B = rows [8194, 40962)
PADA_ROW = 0
PADB_ROW = NROWS - 1             # 40961; idx in B = 32767
REAL = N // NCORES               # 5000 real nodes per core
LN_EPS = 1e-5
ASR_PAD = -1.0e4                 # pad-row a_src => exp(leaky(...)) == 0

_cache = {}


# ---------------------------------------------------------------- host side

def _pack_idx(flat):
    """Flat int list -> [128, n/16] int16 wrapped layout for dma_gather."""
    n = len(flat)
    assert n % 16 == 0
    a = np.asarray(flat)
    assert a.min() >= 0 and a.max() <= 32767, (a.min(), a.max())
    t = a.astype(np.int16).reshape(n // 16, 16).T      # [16, n/16]
    return np.ascontiguousarray(np.tile(t, (8, 1)))    # [128, n/16]


def _perm_from_order(order):
    """order (rank -> orig node) => (row_of, nodes_of_core)."""
    r = np.arange(N)
    blk, pos = r // NCORES, r % NCORES
    core_of_rank = np.where(blk % 2 == 0, pos, NCORES - 1 - pos)
    node_core = np.empty(N, np.int64)
    node_slot = np.empty(N, np.int64)
    node_core[order] = core_of_rank
    node_slot[order] = blk                      # local slot 0..4999
    t = node_slot // P
    p = node_slot % P
    row_of = 1 + (t // GT) * CHUNK + node_core * (GT * P) + (t % GT) * P + p
    nodes_of_core = [order[core_of_rank == c] for c in range(NCORES)]
    return row_of, nodes_of_core


def _preprocess(edge_index):
    src = np.concatenate([edge_index[0].astype(np.int64), np.arange(N)])
    dst = np.concatenate([edge_index[1].astype(np.int64), np.arange(N)])
    indeg = np.bincount(dst, minlength=N)

    # pass 1: degree-sorted; pass 2-3: refine with forced-A counts so tiles
    # (consecutive 1024-rank blocks) are homogeneous in (deg, fa)
    order = np.argsort(-indeg, kind="stable")
    row_of, nodes_of_core = _perm_from_order(order)
    for _ in range(2):
        srow = row_of[src]
        fa_cnt = np.bincount(dst[srow < WINB_BASE], minlength=N)
        order = np.lexsort((-fa_cnt, -indeg))
        row_of, nodes_of_core = _perm_from_order(order)

    e_src_row = row_of[src]
    e_dst_row = row_of[dst]
    eorder = np.argsort(e_dst_row, kind="stable")
    s_src = e_src_row[eorder]
    s_dst = e_dst_row[eorder]
    bounds = np.searchsorted(s_dst, np.arange(NROWS + 1))

    # tile (kA, kB): minimal feasible given per-node forced-A/forced-B counts
    # and degrees: kA >= max fa, kB >= max fb, kA + kB >= max deg.
    node_fa = {}
    node_fb = {}
    node_fl = {}
    faM = np.zeros((NCORES, RTILES), np.int64)
    fbM = np.zeros((NCORES, RTILES), np.int64)
    dgM = np.zeros((NCORES, RTILES), np.int64)
    for c in range(NCORES):
        for t in range(RTILES):
            base = 1 + (t // GT) * CHUNK + c * (GT * P) + (t % GT) * P
            for p in range(P):
                grow = base + p
                lo, hi = bounds[grow], bounds[grow + 1]
                if lo == hi:
                    continue
                srcs = s_src[lo:hi]
                fa = srcs[srcs < WINB_BASE]
                fb = srcs[srcs >= WINA]
                fl = srcs[(srcs >= WINB_BASE) & (srcs < WINA)]
                node_fa[grow] = fa
                node_fb[grow] = fb
                node_fl[grow] = fl
                faM[c, t] = max(faM[c, t], len(fa))
                fbM[c, t] = max(fbM[c, t], len(fb))
                dgM[c, t] = max(dgM[c, t], hi - lo)

    fa_req = np.maximum(faM.max(axis=0), 1)
    fb_req = np.maximum(fbM.max(axis=0), 1)
    tot = np.maximum(dgM.max(axis=0), fa_req + fb_req)
    # split tot into (kA, kB) minimizing gather calls: prefer kA or kB on a
    # multiple-of-8 boundary (each dma_gather covers at most 8 slots)
    sched = []
    for t in range(RTILES):
        A, B, S = int(fa_req[t]), int(fb_req[t]), int(tot[t])
        best = None
        for ka in range(A, S - B + 1):
            kb = S - ka
            calls = -(-ka // 8) - (-kb // 8)
            key = (calls, abs(ka - kb))
            if best is None or key < best[0]:
                best = (key, ka, kb)
        sched.append((best[1], best[2]))
    sched = tuple(sched)

    node_A = {}
    node_B = {}
    tile_of_row = np.zeros(NROWS, np.int64)
    for c in range(NCORES):
        for t in range(RTILES):
            base = 1 + (t // GT) * CHUNK + c * (GT * P) + (t % GT) * P
            tile_of_row[base:base + P] = t
    for grow, fa in node_fa.items():
        t = tile_of_row[grow]
        fb = node_fb[grow]
        fl = node_fl[grow]
        deg = len(fa) + len(fb) + len(fl)
        a_d = int(np.clip(deg - sched[t][1], len(fa), len(fa) + len(fl)))
        node_A[grow] = np.concatenate([fa, fl[: a_d - len(fa)]])
        node_B[grow] = np.concatenate([fb, fl[a_d - len(fa):]]) - WINB_BASE

    idxa_cols, idxb_cols = [], []
    for c in range(NCORES):
        fa_all, fb_all = [], []
        for t in range(RTILES):
            ka, kb = sched[t]
            arrA = np.full((P, ka), PADA_ROW, np.int64)
            arrB = np.full((P, kb), PADB_ROW - WINB_BASE, np.int64)
            base = 1 + (t // GT) * CHUNK + c * (GT * P) + (t % GT) * P
            for p in range(P):
                grow = base + p
                la = node_A.get(grow)
                if la is not None and len(la):
                    arrA[p, : len(la)] = la
                lb = node_B.get(grow)
                if lb is not None and len(lb):
                    arrB[p, : len(lb)] = lb
            fa_all.append(arrA.T.reshape(-1))
            fb_all.append(arrB.T.reshape(-1))
        idxa_cols.append(_pack_idx(np.concatenate(fa_all)))
        idxb_cols.append(_pack_idx(np.concatenate(fb_all)))

    return {
        "sched": sched,
        "nodes_of_core": nodes_of_core,
        "idxa": idxa_cols,
        "idxb": idxb_cols,
    }


def _wext(conv_w, att_src, att_dst):
    """[128, 136] rhs: 0:128 conv_w.T | 128:132 a_src w | 132:136 a_dst w."""
    w = np.zeros((HID, HID + 2 * H), np.float32)
    w[:, :HID] = conv_w.T
    wr = conv_w.reshape(H, C, HID)
    w[:, HID:HID + H] = np.einsum("hc,hcf->fh", att_src, wr)
    w[:, HID + H:] = np.einsum("hc,hcf->fh", att_dst, wr)
    return w


def _bc(vec):
    return np.ascontiguousarray(np.tile(np.asarray(vec, np.float32)[None, :], (P, 1)))


# ---------------------------------------------------------------- bass build

def _build(sched, flags):
    STAGE = int(os.environ.get("GAT_STAGE", "9"))
    g_is1, b_is0, cb_is0, pa_scalar = flags
    nc = bacc.Bacc("TRN2", target_bir_lowering=False, debug=False,
                   enable_asserts=True, num_devices=NCORES, num_swdge_queues=4)
    dt = mybir.dt
    f32, f16, i16, i32 = dt.float32, dt.float16, dt.int16, dt.int32

    nA = 8 * sum(k for k, _ in sched)
    nB = 8 * sum(k for _, k in sched)

    def din(name, shape, dtype):
        return nc.dram_tensor(name, shape, dtype, kind="ExternalInput").ap()

    xT = din("xT", [EMB, SHARD], f16)
    txtT = din("txtT", [TXT, SHARD], f16)
    numT = din("numT", [1, SHARD], f16)
    idxa = din("idxa", [P, nA], i16)
    idxb = din("idxb", [P, nB], i16)
    npwT = din("npwT", [EMB, HID], f16)
    tpwT = din("tpwT", [TXT, HID], f16)
    numwT = din("numwT", [1, HID], f16)
    bias0 = din("bias0", [P, 1], f32)
    prelu0a = din("prelu0a", [P, 1], f32)
    w1ext = din("w1ext", [HID, HID + 2 * H], f16)
    w2ext = din("w2ext", [HID, HID + 2 * H], f16)
    padrow = din("padrow", [1, ROWB], f16)
    cb1 = din("cb1", [P, HID], f32)
    g1 = din("g1", [P, HID], f32)
    bln1 = din("bln1", [P, HID], f32)
    pa1 = din("pa1", [P, HID], f32)
    cb2 = din("cb2", [P, HID], f32)
    g2 = din("g2", [P, HID], f32)
    bln2 = din("bln2", [P, HID], f32)
    pa2 = din("pa2", [P, HID], f32)
    outw = din("outw", [P, HID], f32)
    outb = din("outb", [P, 1], f32)

    out = nc.dram_tensor("out", [SHARD, 1], f32, kind="ExternalOutput").ap()

    # per-layer, per-group collective inputs (local) and full tables (shared)
    cc_g = [[nc.dram_tensor(f"cc{li}_g{g}", [GT * P, ROWB], f16)
             for g in range(NG)] for li in (0, 1)]
    table = [nc.dram_tensor(f"table{i}", [NROWS, ROWB], f16, addr_space="Shared")
             for i in (1, 2)]
    table_ap = [tt.ap() for tt in table]

    def bc_ap(ap, t_count, at=1):
        new = list(map(list, ap.ap))
        new.insert(at, [0, t_count])
        return bass.AP(tensor=ap.tensor, offset=ap.offset, ap=new)

    def app_ap(ap, count):
        new = list(map(list, ap.ap)) + [[0, count]]
        return bass.AP(tensor=ap.tensor, offset=ap.offset, ap=new)

    qctr = [0]

    def next_q():
        qctr[0] += 1
        return qctr[0] % 4

    with tile.TileContext(nc) as tc:
        consts = tc.alloc_tile_pool(name="consts", bufs=1)
        persist = tc.alloc_tile_pool(name="persist", bufs=1)
        io = tc.alloc_tile_pool(name="io", bufs=2)
        work = tc.alloc_tile_pool(name="work", bufs=2)
        ep = tc.alloc_tile_pool(name="ep", bufs=2)
        psA = tc.alloc_tile_pool(name="psA", bufs=2, space="PSUM")
        psB = tc.alloc_tile_pool(name="psB", bufs=2, space="PSUM")
        psC = tc.alloc_tile_pool(name="psC", bufs=2, space="PSUM")
        psT = tc.alloc_tile_pool(name="psT", bufs=2, space="PSUM")

        _ld_n = [0]

        def ld(ap_in, shape, dtype, pool=consts):
            _ld_n[0] += 1
            nm = f"const{_ld_n[0]}"
            t = pool.tile(shape, dtype, name=nm, tag=nm)
            nc.sync.dma_start(out=t[:], in_=ap_in)
            return t

        sb_idxa = ld(idxa, [P, nA], i16)
        sb_idxb = ld(idxb, [P, nB], i16)
        sb_npwT = ld(npwT, [EMB, HID], f16)
        sb_tpwT = [ld(ch, [P, HID], f16) for ch in
                   (tpwT[0:P, :], tpwT[P:2 * P, :], tpwT[2 * P:3 * P, :])]
        sb_numwT = ld(numwT, [1, HID], f16)
        sb_bias0 = ld(bias0, [P, 1], f32)
        sb_pr0a = ld(prelu0a, [P, 1], f32)
        sb_wext = [ld(w1ext, [HID, HID + 2 * H], f16),
                   ld(w2ext, [HID, HID + 2 * H], f16)]
        sb_cb = [ld(cb1, [P, HID], f32), ld(cb2, [P, HID], f32)]
        sb_g = [ld(g1, [P, HID], f32), ld(g2, [P, HID], f32)]
        sb_bln = [ld(bln1, [P, HID], f32), ld(bln2, [P, HID], f32)]
        sb_pa = [ld(pa1, [P, HID], f32), ld(pa2, [P, HID], f32)]
        sb_outw = ld(outw, [P, HID], f32)
        sb_outb = ld(outb, [P, 1], f32)

        ident16 = consts.tile([P, P], f16)
        make_identity(nc, ident16[:])
        identf = consts.tile([P, P], f32)
        make_identity(nc, identf[:])
        eps_t = consts.tile([P, 1], f32)
        nc.vector.memset(eps_t[:], LN_EPS)

        hT = [persist.tile([P, SHARD], f16, tag=f"hT{i}", name=f"hT{i}")
              for i in range(1)]
        # conv1 output kept as per-tile tiles so conv2's table build can
        # start before the whole conv1 edge phase finishes
        h1t = [persist.tile([P, P], f16, tag=f"h1t{t}", name=f"h1t{t}")
               for t in range(RTILES)]
        adst_all = persist.tile([P, 2, RTILES, H], f16)

        # pad rows: every core writes its own table copies locally
        for li in range(2):
            nc.sync.dma_start(out=table_ap[li][PADA_ROW:PADA_ROW + 1, :],
                              in_=padrow[0:1, :])
            nc.sync.dma_start(out=table_ap[li][PADB_ROW:PADB_ROW + 1, :],
                              in_=padrow[0:1, :])

        def build_tile(li, t, lhsT_t):
            """Emit table-row build for tile t of layer li."""
            ps = psB.tile([P, HID + 2 * H], f32, tag="tb")
            nc.tensor.matmul(ps[:], lhsT=lhsT_t,
                             rhs=sb_wext[li][:], start=True, stop=True)
            trow = work.tile([P, ROWB], f16, tag="trow")
            nc.scalar.copy(out=trow[:, 0:HID + H], in_=ps[:, 0:HID + H])
            g = t // GT
            r0 = (t % GT) * P
            nc.sync.dma_start(out=cc_g[li][g].ap()[r0:r0 + P, :], in_=trow[:])
            nc.vector.tensor_copy(out=adst_all[:, li, t, :],
                                  in_=ps[:, HID + H:HID + 2 * H])

        def fire_group(li, g):
            """AllGather group g of layer li into the shared table."""
            row0 = 1 + g * CHUNK
            nc.gpsimd.collective_compute(
                "AllGather", mybir.AluOpType.bypass,
                replica_groups=[list(range(NCORES))],
                ins=[cc_g[li][g].ap().opt()],
                outs=[table_ap[li][row0:row0 + CHUNK, :].opt()],
            )

        # ---- stage 1: h0T = prelu0(proj(x, txt, num) + bias0), transposed;
        # conv1 table build + group collectives interleaved per 512-col chunk
        col = 0
        built = 0
        while col < SHARD:
            cw = min(512, SHARD - col)
            sl = slice(col, col + cw)
            x_t = io.tile([P, cw], f16, tag="x")
            nc.sync.dma_start(out=x_t[:], in_=xT[:, sl])
            tx_t = [io.tile([P, cw], f16, tag=f"tx{k}", name=f"tx{k}")
                    for k in range(3)]
            for k in range(3):
                nc.sync.dma_start(out=tx_t[k][:], in_=txtT[k * P:(k + 1) * P, sl])
            nm_t = io.tile([1, cw], f16, tag="nm")
            nc.sync.dma_start(out=nm_t[:], in_=numT[0:1, sl])

            ps = psA.tile([P, cw], f32, tag="ps1")
            nc.tensor.matmul(ps[:], lhsT=sb_npwT[:], rhs=x_t[:],
                             start=True, stop=False)
            for k in range(3):
                nc.tensor.matmul(ps[:], lhsT=sb_tpwT[k][:], rhs=tx_t[k][:],
                                 start=False, stop=False)
            nc.tensor.matmul(ps[:], lhsT=sb_numwT[:], rhs=nm_t[:],
                             start=False, stop=True)
            nc.scalar.activation(out=hT[0][:, sl], in_=ps[:],
                                 func=mybir.ActivationFunctionType.Prelu,
                                 bias=sb_bias0[:], alpha=sb_pr0a[:])
            col += cw
            # table build for completed tiles
            done = min(col // P, RTILES)
            while built < done:
                t = built
                build_tile(0, t, hT[0][:, t * P:(t + 1) * P])
                built += 1
                if built % GT == 0:
                    fire_group(0, built // GT - 1)

        # ---- conv layers
        nconv = 0 if STAGE <= 1 else (2 if STAGE >= 9 else 1)
        for li in range(nconv):
            if STAGE <= 2:
                continue
            oa = ob = 0
            for t in range(RTILES):
                ka, kb = sched[t]
                T = ka + kb
                T3 = 3 * ((T + 2) // 3)
                G = work.tile([P, T, ROWB], f16, tag="G",
                              bufs=int(os.environ.get("GAT_GBUFS", "3")))
                for g0 in range(0, ka, 8):
                    n = min(8, ka - g0)
                    nc.gpsimd.dma_gather(
                        G[:, g0:g0 + n, :], table_ap[li][0:WINA, :],
                        sb_idxa[:, oa + g0 * 8:oa + (g0 + n) * 8],
                        n * P, n * P, ROWB, queue_num=next_q())
                for g0 in range(0, kb, 8):
                    n = min(8, kb - g0)
                    nc.gpsimd.dma_gather(
                        G[:, ka + g0:ka + g0 + n, :],
                        table_ap[li][WINB_BASE:NROWS, :],
                        sb_idxb[:, ob + g0 * 8:ob + (g0 + n) * 8],
                        n * P, n * P, ROWB, queue_num=next_q())
                oa += ka * 8
                ob += kb * 8
                if STAGE == 3:
                    continue

                alph = work.tile([P, T, H], f16, tag="alph")
                nc.vector.tensor_tensor(out=alph[:],
                                        in0=G[:, :, HID:HID + H],
                                        in1=bc_ap(adst_all[:, li, t, :], T),
                                        op=mybir.AluOpType.add)
                # leaky relu on DVE: max(a, 0.2a)
                nc.vector.scalar_tensor_tensor(out=alph[:], in0=alph[:],
                                               scalar=0.2, in1=alph[:],
                                               op0=mybir.AluOpType.mult,
                                               op1=mybir.AluOpType.max)
                RHS = work.tile([P, T3, HID + H], f16, tag="RHS")
                if T3 > T:
                    nc.vector.memset(RHS[:, T:T3, :], 0)
                nc.scalar.activation(out=RHS[:, 0:T, HID:HID + H], in_=alph[:],
                                     func=mybir.ActivationFunctionType.Exp)
                ex_b = RHS[:, 0:T, HID:HID + H]
                nc.vector.tensor_tensor(
                    out=RHS[:, 0:T, 0:HID].rearrange("p t (h c) -> p t h c", h=H),
                    in0=G[:, :, 0:HID].rearrange("p t (h c) -> p t h c", h=H),
                    in1=app_ap(ex_b, C), op=mybir.AluOpType.mult)

                if STAGE == 36:
                    continue
                ps2 = psC.tile([P, 3, HID + H], f32, tag="cv")
                ng = T3 // 3
                for g in range(ng):
                    nc.tensor.matmul(ps2[:], lhsT=ident16[:],
                                     rhs=RHS[:, 3 * g:3 * g + 3, :],
                                     start=(g == 0), stop=(g == ng - 1))
                nm2 = ep.tile([P, HID + H], f32, tag="nm2")
                ps2v = ps2[:]
                ps2_sw = bass.AP(
                    tensor=ps2v.tensor, offset=ps2v.offset,
                    ap=[list(ps2v.ap[0]), [1, HID + H], [HID + H, 3]])
                nc.vector.reduce_sum(out=nm2[:], in_=ps2_sw,
                                     axis=mybir.AxisListType.X)

                den = ep.tile([P, H], f32, tag="den")
                if t == RTILES - 1:
                    # +eps so empty dst rows (no edges, tail of the last
                    # tile) yield 0/eps = 0, not NaN
                    nc.vector.tensor_scalar_add(out=nm2[:, HID:HID + H],
                                                in0=nm2[:, HID:HID + H],
                                                scalar1=1e-30)
                nc.vector.reciprocal(out=den[:], in_=nm2[:, HID:HID + H])
                hb = ep.tile([P, HID], f32, tag="hb")
                nc.vector.tensor_tensor(
                    out=hb[:].rearrange("p (h c) -> p h c", h=H),
                    in0=nm2[:, 0:HID].rearrange("p (h c) -> p h c", h=H),
                    in1=app_ap(den[:], C), op=mybir.AluOpType.mult)
                if not cb_is0:
                    nc.vector.tensor_tensor(out=hb[:], in0=hb[:], in1=sb_cb[li][:],
                                            op=mybir.AluOpType.add)
                stats = ep.tile([P, nc.vector.BN_STATS_DIM], f32, tag="st")
                nc.vector.bn_stats(out=stats[:], in_=hb[:])
                mv = ep.tile([P, nc.vector.BN_AGGR_DIM], f32, tag="mv")
                nc.vector.bn_aggr(out=mv[:], in_=stats[:])
                sq = ep.tile([P, 1], f32, tag="sq")
                nc.scalar.activation(out=sq[:], in_=mv[:, 1:2],
                                     func=mybir.ActivationFunctionType.Sqrt,
                                     bias=eps_t[:])
                nc.vector.reciprocal(out=sq[:], in_=sq[:])
                if g_is1 and b_is0 and pa_scalar is not None:
                    # fused (hb - mean) * rstd then prelu, on the scalar engine
                    nbias = ep.tile([P, 1], f32, tag="nb")
                    nc.vector.scalar_tensor_tensor(out=nbias[:], in0=mv[:, 0:1],
                                                   scalar=-1.0, in1=sq[:],
                                                   op0=mybir.AluOpType.mult,
                                                   op1=mybir.AluOpType.mult)
                    hb2 = ep.tile([P, HID], f32, tag="hb2")
                    nc.scalar.activation(out=hb2[:], in_=hb[:],
                                         func=mybir.ActivationFunctionType.Prelu,
                                         bias=nbias[:], scale=sq[:],
                                         alpha=float(pa_scalar))
                    hb = hb2
                else:
                    # hb = (hb - mean) * rstd
                    nc.vector.tensor_scalar(out=hb[:], in0=hb[:], scalar1=mv[:, 0:1],
                                            scalar2=sq[:, 0:1],
                                            op0=mybir.AluOpType.subtract,
                                            op1=mybir.AluOpType.mult)
                    if not g_is1:
                        nc.vector.tensor_tensor(out=hb[:], in0=hb[:], in1=sb_g[li][:],
                                                op=mybir.AluOpType.mult)
                    if not b_is0:
                        nc.vector.tensor_tensor(out=hb[:], in0=hb[:], in1=sb_bln[li][:],
                                                op=mybir.AluOpType.add)
                    # prelu
                    t2 = ep.tile([P, HID], f32, tag="t2")
                    if pa_scalar is not None:
                        nc.vector.tensor_scalar(out=t2[:], in0=hb[:], scalar1=0.0,
                                                scalar2=float(pa_scalar),
                                                op0=mybir.AluOpType.min,
                                                op1=mybir.AluOpType.mult)
                    else:
                        nc.vector.tensor_scalar(out=t2[:], in0=hb[:], scalar1=0.0,
                                                scalar2=None, op0=mybir.AluOpType.min)
                        nc.vector.tensor_tensor(out=t2[:], in0=t2[:], in1=sb_pa[li][:],
                                                op=mybir.AluOpType.mult)
                    nc.vector.scalar_tensor_tensor(out=hb[:], in0=hb[:], scalar=0.0,
                                                   in1=t2[:], op0=mybir.AluOpType.max,
                                                   op1=mybir.AluOpType.add)

                if li == 0:
                    pst = psT.tile([P, P], f32, tag="tr")
                    nc.tensor.transpose(out=pst[:], in_=hb[:], identity=identf[:])
                    nc.vector.tensor_copy(out=h1t[t][:], in_=pst[:])
                    # conv2 table build rides the conv1 edge loop so its
                    # group collectives overlap conv1 edge processing
                    build_tile(1, t, h1t[t][:])
                    if (t + 1) % GT == 0:
                        fire_group(1, (t + 1) // GT - 1)
                else:
                    om = ep.tile([P, HID], f32, tag="om")
                    nc.vector.tensor_tensor(out=om[:], in0=hb[:], in1=sb_outw[:],
                                            op=mybir.AluOpType.mult)
                    ov = ep.tile([P, 1], f32, tag="ov")
                    nc.vector.reduce_sum(out=ov[:], in_=om[:],
                                         axis=mybir.AxisListType.X)
                    nc.vector.tensor_scalar_add(out=ov[:], in0=ov[:],
                                                scalar1=sb_outb[:, 0:1])
                    nc.sync.dma_start(out=out[t * P:(t + 1) * P, :], in_=ov[:])

        for p in (psT, psC, psB, psA, ep, work, io, persist, consts):
            p.release()

    nc.compile()
    return nc


# ---------------------------------------------------------------- entry point

def kernel(x, num_x, num_mask, txt_x, txt_mask, edge_index,
           num_proj_w, num_proj_b, txt_proj_w, txt_proj_b,
           node_proj_w, node_proj_b, prelu0_a,
           conv1_w, att_src1, att_dst1, bias1, norm1_g, norm1_b, prelu1_a,
           conv2_w, att_src2, att_dst2, bias2, norm2_g, norm2_b, prelu2_a,
           out_w, out_b, _trace=False):
    x = np.asarray(x, np.float32)
    edge_index = np.asarray(edge_index)

    g_is1 = bool(np.all(norm1_g == 1) and np.all(norm2_g == 1))
    b_is0 = bool(np.all(norm1_b == 0) and np.all(norm2_b == 0))
    cb_is0 = bool(np.all(np.asarray(bias1) == 0) and np.all(np.asarray(bias2) == 0))
    pa1a = np.asarray(prelu1_a, np.float32)
    pa2a = np.asarray(prelu2_a, np.float32)
    pa_scalar = float(pa1a[0]) if (np.all(pa1a == pa1a[0])
                                   and np.all(pa2a == pa1a[0])) else None
    flags = (g_is1, b_is0, cb_is0, pa_scalar)

    pre_key = (hash(edge_index.tobytes()), flags)
    if pre_key in _cache:
        pre, nc = _cache[pre_key]
    else:
        pre = _preprocess(edge_index)
        nc = _build(pre["sched"], flags)
        _cache[pre_key] = (pre, nc)

    numv = (np.asarray(num_x, np.float32)[:, 0] * np.asarray(num_mask, np.float32))
    txtv = np.asarray(txt_x, np.float32) * np.asarray(txt_mask, np.float32)[:, None]
    bias0 = (np.asarray(num_proj_b) + np.asarray(txt_proj_b)
             + np.asarray(node_proj_b)).astype(np.float32)

    padrow = np.zeros((1, ROWB), np.float32)
    padrow[0, HID:HID + H] = ASR_PAD

    shared = {
        "npwT": np.ascontiguousarray(np.asarray(node_proj_w, np.float32).T).astype(F16),
        "tpwT": np.ascontiguousarray(np.asarray(txt_proj_w, np.float32).T).astype(F16),
        "numwT": np.ascontiguousarray(np.asarray(num_proj_w, np.float32).T).astype(F16),
        "bias0": bias0[:, None],
        "prelu0a": np.asarray(prelu0_a, np.float32)[:, None],
        "w1ext": _wext(np.asarray(conv1_w, np.float32),
                       np.asarray(att_src1, np.float32),
                       np.asarray(att_dst1, np.float32)).astype(F16),
        "w2ext": _wext(np.asarray(conv2_w, np.float32),
                       np.asarray(att_src2, np.float32),
                       np.asarray(att_dst2, np.float32)).astype(F16),
        "padrow": padrow.astype(F16),
        "cb1": _bc(bias1), "g1": _bc(norm1_g), "bln1": _bc(norm1_b), "pa1": _bc(prelu1_a),
        "cb2": _bc(bias2), "g2": _bc(norm2_g), "bln2": _bc(norm2_b), "pa2": _bc(prelu2_a),
        "outw": _bc(np.asarray(out_w, np.float32)[0]),
        "outb": np.full((P, 1), np.asarray(out_b, np.float32)[0], np.float32),
    }

    in_maps = []
    for c in range(NCORES):
        nodes = pre["nodes_of_core"][c]
        xTa = np.zeros((EMB, SHARD), np.float32)
        xTa[:, :REAL] = x[nodes].T
        txtTa = np.zeros((TXT, SHARD), np.float32)
        txtTa[:, :REAL] = txtv[nodes].T
        numTa = np.zeros((1, SHARD), np.float32)
        numTa[0, :REAL] = numv[nodes]
        m = dict(shared)
        m["xT"] = xTa.astype(F16)
        m["txtT"] = txtTa.astype(F16)
        m["numT"] = numTa.astype(F16)
        m["idxa"] = pre["idxa"][c]
        m["idxb"] = pre["idxb"][c]
        in_maps.append(m)

    res = run_bass_kernel_spmd(nc, in_maps, core_ids=list(range(NCORES)),
                               trace=_trace)
    out_full = np.zeros(N, np.float32)
    for c in range(NCORES):
        out_full[pre["nodes_of_core"][c]] = res.results[c]["out"][:REAL, 0]
    if _trace:
        kernel._last_exec_ns = res.exec_time_ns
        kernel._last_trace = res.instructions_and_trace
    return out_full


# revision 15
# speedup vs baseline: 1.2626x; 1.0716x over previous
"""Trainium2 Bass kernel for a 2-layer GAT network (nn_GATNet).

Sharding: nodes permuted host-side (degree-sorted, snake-dealt across 8
cores). Table layout is GROUP-major: 5 groups x 8 tiles; within a group,
8 cores x 8 tiles x 128 rows, so each group's AllGather writes one
contiguous table slice. Row 0 / row NROWS-1 are pad rows (written locally
by every core, never AllGathered) whose a_src = -1e4 makes exp == 0 for
pad slots. Per conv: each core builds its table tiles (xh | a_src | junk
as 512B f16 rows via one matmul per tile; same matmul yields a_dst kept
in SBUF), fires a chunked AllGather per finished group (overlapping the
next group's build and the previous phase), then processes its
destination-bucketed edges: dst node (t,p) owns partition p of tile t,
incoming edges in slot columns fetched by dma_gather (int16 indices =>
two overlapping <=32768-row windows A/B; 1024 idx max per gather; 4 SWDGE
queues round-robin). Softmax skips max-subtraction (alphas are O(1)).
Slot accumulation is an identity-matmul into PSUM, 3 slots per matmul,
combined on DVE. Epilogue: normalize, layernorm, prelu, PE-transpose back
to [feat, node]; conv2's table build is interleaved into conv1's edge
loop so its AllGathers hide behind conv1 edge processing.
"""

import os
import numpy as np
import ml_dtypes

import concourse.bacc as bacc
import concourse.tile as tile
import concourse.bass as bass
import concourse.mybir as mybir
from concourse.bass_utils import run_bass_kernel_spmd
from concourse.masks import make_identity

F16 = np.float16

N, E = 40000, 640000
EMB, HID, H, TXT = 128, 128, 4, 384
C = HID // H
NCORES = 8
P = 128
RTILES = 40                      # real node tiles per core
GT = int(os.environ.get("GAT_GT", "40"))  # tiles per collective group
NG = RTILES // GT                # groups (collectives per layer)
CHUNK = NCORES * GT * P          # table rows per group (8192)
SHARD = RTILES * P               # 5120 real rows per core
NROWS = 2 + NCORES * SHARD      # 40962: row 0 = padA, last = padB
ROWB = 256                       # f16 elements per table row (512 B)
WINA = 32767                     # window A = rows [0, 32767)
WINB_BASE = NROWS - 32768        # 8194; window B = rows [8194, 40962)
PADA_ROW = 0
PADB_ROW = NROWS - 1             # 40961; idx in B = 32767
REAL = N // NCORES               # 5000 real nodes per core
LN_EPS = 1e-5
ASR_PAD = -1.0e4                 # pad-row a_src => exp(leaky(...)) == 0

_cache = {}


# ---------------------------------------------------------------- host side

def _pack_idx(flat):
    """Flat int list -> [128, n/16] int16 wrapped layout for dma_gather."""
    n = len(flat)
    assert n % 16 == 0
    a = np.asarray(flat)
    assert a.min() >= 0 and a.max() <= 32767, (a.min(), a.max())
    t = a.astype(np.int16).reshape(n // 16, 16).T      # [16, n/16]
    return np.ascontiguousarray(np.tile(t, (8, 1)))    # [128, n/16]


def _perm_from_order(order):
    """order (rank -> orig node) => (row_of, nodes_of_core)."""
    r = np.arange(N)
    blk, pos = r // NCORES, r % NCORES
    core_of_rank = np.where(blk % 2 == 0, pos, NCORES - 1 - pos)
    node_core = np.empty(N, np.int64)
    node_slot = np.empty(N, np.int64)
    node_core[order] = core_of_rank
    node_slot[order] = blk                      # local slot 0..4999
    t = node_slot // P
    p = node_slot % P
    row_of = 1 + (t // GT) * CHUNK + node_core * (GT * P) + (t % GT) * P + p
    nodes_of_core = [order[core_of_rank == c] for c in range(NCORES)]
    return row_of, nodes_of_core


def _preprocess(edge_index):
    # self-loops are NOT gathered: their contribution is computed from the
    # local (non-AllGathered) table rows inside the edge loop
    src = edge_index[0].astype(np.int64)
    dst = edge_index[1].astype(np.int64)
    indeg = np.bincount(dst, minlength=N)

    # pass 1: degree-sorted; pass 2-3: refine with forced-A counts so tiles
    # (consecutive 1024-rank blocks) are homogeneous in (deg, fa)
    order = np.argsort(-indeg, kind="stable")
    row_of, nodes_of_core = _perm_from_order(order)
    for _ in range(2):
        srow = row_of[src]
        fa_cnt = np.bincount(dst[srow < WINB_BASE], minlength=N)
        order = np.lexsort((-fa_cnt, -indeg))
        row_of, nodes_of_core = _perm_from_order(order)

    e_src_row = row_of[src]
    e_dst_row = row_of[dst]
    eorder = np.argsort(e_dst_row, kind="stable")
    s_src = e_src_row[eorder]
    s_dst = e_dst_row[eorder]
    bounds = np.searchsorted(s_dst, np.arange(NROWS + 1))

    # tile (kA, kB): minimal feasible given per-node forced-A/forced-B counts
    # and degrees: kA >= max fa, kB >= max fb, kA + kB >= max deg.
    node_fa = {}
    node_fb = {}
    node_fl = {}
    faM = np.zeros((NCORES, RTILES), np.int64)
    fbM = np.zeros((NCORES, RTILES), np.int64)
    dgM = np.zeros((NCORES, RTILES), np.int64)
    for c in range(NCORES):
        for t in range(RTILES):
            base = 1 + (t // GT) * CHUNK + c * (GT * P) + (t % GT) * P
            for p in range(P):
                grow = base + p
                lo, hi = bounds[grow], bounds[grow + 1]
                if lo == hi:
                    continue
                srcs = s_src[lo:hi]
                fa = srcs[srcs < WINB_BASE]
                fb = srcs[srcs >= WINA]
                fl = srcs[(srcs >= WINB_BASE) & (srcs < WINA)]
                node_fa[grow] = fa
                node_fb[grow] = fb
                node_fl[grow] = fl
                faM[c, t] = max(faM[c, t], len(fa))
                fbM[c, t] = max(fbM[c, t], len(fb))
                dgM[c, t] = max(dgM[c, t], hi - lo)

    fa_req = np.maximum(faM.max(axis=0), 1)
    fb_req = np.maximum(fbM.max(axis=0), 1)
    tot = np.maximum(dgM.max(axis=0), fa_req + fb_req)
    # split tot into (kA, kB) minimizing gather calls: prefer kA or kB on a
    # multiple-of-8 boundary (each dma_gather covers at most 8 slots)
    sched = []
    for t in range(RTILES):
        A, B, S = int(fa_req[t]), int(fb_req[t]), int(tot[t])
        best = None
        for ka in range(A, S - B + 1):
            kb = S - ka
            calls = -(-ka // 8) - (-kb // 8)
            key = (calls, abs(ka - kb))
            if best is None or key < best[0]:
                best = (key, ka, kb)
        sched.append((best[1], best[2]))
    sched = tuple(sched)

    node_A = {}
    node_B = {}
    tile_of_row = np.zeros(NROWS, np.int64)
    for c in range(NCORES):
        for t in range(RTILES):
            base = 1 + (t // GT) * CHUNK + c * (GT * P) + (t % GT) * P
            tile_of_row[base:base + P] = t
    for grow, fa in node_fa.items():
        t = tile_of_row[grow]
        fb = node_fb[grow]
        fl = node_fl[grow]
        deg = len(fa) + len(fb) + len(fl)
        a_d = int(np.clip(deg - sched[t][1], len(fa), len(fa) + len(fl)))
        node_A[grow] = np.concatenate([fa, fl[: a_d - len(fa)]])
        node_B[grow] = np.concatenate([fb, fl[a_d - len(fa):]]) - WINB_BASE

    idxa_cols, idxb_cols = [], []
    for c in range(NCORES):
        fa_all, fb_all = [], []
        for t in range(RTILES):
            ka, kb = sched[t]
            arrA = np.full((P, ka), PADA_ROW, np.int64)
            arrB = np.full((P, kb), PADB_ROW - WINB_BASE, np.int64)
            base = 1 + (t // GT) * CHUNK + c * (GT * P) + (t % GT) * P
            for p in range(P):
                grow = base + p
                la = node_A.get(grow)
                if la is not None and len(la):
                    arrA[p, : len(la)] = la
                lb = node_B.get(grow)
                if lb is not None and len(lb):
                    arrB[p, : len(lb)] = lb
            fa_all.append(arrA.T.reshape(-1))
            fb_all.append(arrB.T.reshape(-1))
        idxa_cols.append(_pack_idx(np.concatenate(fa_all)))
        idxb_cols.append(_pack_idx(np.concatenate(fb_all)))

    return {
        "sched": sched,
        "nodes_of_core": nodes_of_core,
        "idxa": idxa_cols,
        "idxb": idxb_cols,
    }


def _wext(conv_w, att_src, att_dst):
    """[128, 136] rhs: 0:128 conv_w.T | 128:132 a_src w | 132:136 a_dst w."""
    w = np.zeros((HID, HID + 2 * H), np.float32)
    w[:, :HID] = conv_w.T
    wr = conv_w.reshape(H, C, HID)
    w[:, HID:HID + H] = np.einsum("hc,hcf->fh", att_src, wr)
    w[:, HID + H:] = np.einsum("hc,hcf->fh", att_dst, wr)
    return w


def _bc(vec):
    return np.ascontiguousarray(np.tile(np.asarray(vec, np.float32)[None, :], (P, 1)))


# ---------------------------------------------------------------- bass build

def _build(sched, flags):
    STAGE = int(os.environ.get("GAT_STAGE", "9"))
    g_is1, b_is0, cb_is0, pa_scalar = flags
    nc = bacc.Bacc("TRN2", target_bir_lowering=False, debug=False,
                   enable_asserts=True, num_devices=NCORES, num_swdge_queues=4)
    dt = mybir.dt
    f32, f16, i16, i32 = dt.float32, dt.float16, dt.int16, dt.int32

    nA = 8 * sum(k for k, _ in sched)
    nB = 8 * sum(k for _, k in sched)

    def din(name, shape, dtype):
        return nc.dram_tensor(name, shape, dtype, kind="ExternalInput").ap()

    xT = din("xT", [EMB, SHARD], f16)
    txtT = din("txtT", [TXT, SHARD], f16)
    numT = din("numT", [1, SHARD], f16)
    idxa = din("idxa", [P, nA], i16)
    idxb = din("idxb", [P, nB], i16)
    npwT = din("npwT", [EMB, HID], f16)
    tpwT = din("tpwT", [TXT, HID], f16)
    numwT = din("numwT", [1, HID], f16)
    bias0 = din("bias0", [P, 1], f32)
    prelu0a = din("prelu0a", [P, 1], f32)
    w1ext = din("w1ext", [HID, HID + 2 * H], f16)
    w2ext = din("w2ext", [HID, HID + 2 * H], f16)
    padrow = din("padrow", [1, ROWB], f16)
    cb1 = din("cb1", [P, HID], f32)
    g1 = din("g1", [P, HID], f32)
    bln1 = din("bln1", [P, HID], f32)
    pa1 = din("pa1", [P, HID], f32)
    cb2 = din("cb2", [P, HID], f32)
    g2 = din("g2", [P, HID], f32)
    bln2 = din("bln2", [P, HID], f32)
    pa2 = din("pa2", [P, HID], f32)
    outw = din("outw", [P, HID], f32)
    outb = din("outb", [P, 1], f32)

    out = nc.dram_tensor("out", [SHARD, 1], f32, kind="ExternalOutput").ap()

    # per-layer, per-group collective inputs (local) and full tables (shared)
    cc_g = [[nc.dram_tensor(f"cc{li}_g{g}", [GT * P, ROWB], f16)
             for g in range(NG)] for li in (0, 1)]
    table = [nc.dram_tensor(f"table{i}", [NROWS, ROWB], f16, addr_space="Shared")
             for i in (1, 2)]
    table_ap = [tt.ap() for tt in table]

    def bc_ap(ap, t_count, at=1):
        new = list(map(list, ap.ap))
        new.insert(at, [0, t_count])
        return bass.AP(tensor=ap.tensor, offset=ap.offset, ap=new)

    def app_ap(ap, count):
        new = list(map(list, ap.ap)) + [[0, count]]
        return bass.AP(tensor=ap.tensor, offset=ap.offset, ap=new)

    qctr = [0]

    def next_q():
        qctr[0] += 1
        return qctr[0] % 4

    with tile.TileContext(nc) as tc:
        consts = tc.alloc_tile_pool(name="consts", bufs=1)
        persist = tc.alloc_tile_pool(name="persist", bufs=1)
        io = tc.alloc_tile_pool(name="io", bufs=2)
        work = tc.alloc_tile_pool(name="work", bufs=2)
        ep = tc.alloc_tile_pool(name="ep", bufs=2)
        psA = tc.alloc_tile_pool(name="psA", bufs=2, space="PSUM")
        psB = tc.alloc_tile_pool(name="psB", bufs=2, space="PSUM")
        psC = tc.alloc_tile_pool(name="psC", bufs=2, space="PSUM")
        psT = tc.alloc_tile_pool(name="psT", bufs=2, space="PSUM")

        _ld_n = [0]

        def ld(ap_in, shape, dtype, pool=consts):
            _ld_n[0] += 1
            nm = f"const{_ld_n[0]}"
            t = pool.tile(shape, dtype, name=nm, tag=nm)
            nc.sync.dma_start(out=t[:], in_=ap_in)
            return t

        sb_idxa = ld(idxa, [P, nA], i16)
        sb_idxb = ld(idxb, [P, nB], i16)
        sb_npwT = ld(npwT, [EMB, HID], f16)
        sb_tpwT = [ld(ch, [P, HID], f16) for ch in
                   (tpwT[0:P, :], tpwT[P:2 * P, :], tpwT[2 * P:3 * P, :])]
        sb_numwT = ld(numwT, [1, HID], f16)
        sb_bias0 = ld(bias0, [P, 1], f32)
        sb_pr0a = ld(prelu0a, [P, 1], f32)
        sb_wext = [ld(w1ext, [HID, HID + 2 * H], f16),
                   ld(w2ext, [HID, HID + 2 * H], f16)]
        sb_cb = [ld(cb1, [P, HID], f32), ld(cb2, [P, HID], f32)]
        sb_g = [ld(g1, [P, HID], f32), ld(g2, [P, HID], f32)]
        sb_bln = [ld(bln1, [P, HID], f32), ld(bln2, [P, HID], f32)]
        sb_pa = [ld(pa1, [P, HID], f32), ld(pa2, [P, HID], f32)]
        sb_outw = ld(outw, [P, HID], f32)
        sb_outb = ld(outb, [P, 1], f32)

        ident16 = consts.tile([P, P], f16)
        make_identity(nc, ident16[:])
        identf = consts.tile([P, P], f32)
        make_identity(nc, identf[:])
        eps_t = consts.tile([P, 1], f32)
        nc.vector.memset(eps_t[:], LN_EPS)

        hT = [persist.tile([P, SHARD], f16, tag=f"hT{i}", name=f"hT{i}")
              for i in range(1)]
        # conv1 output kept as per-tile tiles so conv2's table build can
        # start before the whole conv1 edge phase finishes
        h1t = [persist.tile([P, P], f16, tag=f"h1t{t}", name=f"h1t{t}")
               for t in range(RTILES)]
        # local copy of this core's table rows (xh | a_src | a_dst): feeds
        # the cc DMA and the non-gathered self-loop slot in the edge loop
        xh_loc = persist.tile([P, 2, RTILES, HID + 2 * H], f16)

        # pad rows: every core writes its own table copies locally
        for li in range(2):
            nc.sync.dma_start(out=table_ap[li][PADA_ROW:PADA_ROW + 1, :],
                              in_=padrow[0:1, :])
            nc.sync.dma_start(out=table_ap[li][PADB_ROW:PADB_ROW + 1, :],
                              in_=padrow[0:1, :])

        def build_tile(li, t, lhsT_t):
            """Emit table-row build for tile t of layer li."""
            ps = psB.tile([P, HID + 2 * H], f32, tag="tb")
            nc.tensor.matmul(ps[:], lhsT=lhsT_t,
                             rhs=sb_wext[li][:], start=True, stop=True)
            nc.scalar.copy(out=xh_loc[:, li, t, :], in_=ps[:])
            g = t // GT
            r0 = (t % GT) * P
            nc.sync.dma_start(out=cc_g[li][g].ap()[r0:r0 + P, 0:HID + H],
                              in_=xh_loc[:, li, t, 0:HID + H])

        def fire_group(li, g):
            """AllGather group g of layer li into the shared table."""
            row0 = 1 + g * CHUNK
            nc.gpsimd.collective_compute(
                "AllGather", mybir.AluOpType.bypass,
                replica_groups=[list(range(NCORES))],
                ins=[cc_g[li][g].ap().opt()],
                outs=[table_ap[li][row0:row0 + CHUNK, :].opt()],
            )

        # ---- stage 1: h0T = prelu0(proj(x, txt, num) + bias0), transposed;
        # conv1 table build + group collectives interleaved per 512-col chunk
        col = 0
        built = 0
        while col < SHARD:
            cw = min(512, SHARD - col)
            sl = slice(col, col + cw)
            x_t = io.tile([P, cw], f16, tag="x")
            nc.sync.dma_start(out=x_t[:], in_=xT[:, sl])
            tx_t = [io.tile([P, cw], f16, tag=f"tx{k}", name=f"tx{k}")
                    for k in range(3)]
            for k in range(3):
                nc.sync.dma_start(out=tx_t[k][:], in_=txtT[k * P:(k + 1) * P, sl])
            nm_t = io.tile([1, cw], f16, tag="nm")
            nc.sync.dma_start(out=nm_t[:], in_=numT[0:1, sl])

            ps = psA.tile([P, cw], f32, tag="ps1")
            nc.tensor.matmul(ps[:], lhsT=sb_npwT[:], rhs=x_t[:],
                             start=True, stop=False)
            for k in range(3):
                nc.tensor.matmul(ps[:], lhsT=sb_tpwT[k][:], rhs=tx_t[k][:],
                                 start=False, stop=False)
            nc.tensor.matmul(ps[:], lhsT=sb_numwT[:], rhs=nm_t[:],
                             start=False, stop=True)
            nc.scalar.activation(out=hT[0][:, sl], in_=ps[:],
                                 func=mybir.ActivationFunctionType.Prelu,
                                 bias=sb_bias0[:], alpha=sb_pr0a[:])
            col += cw
            # table build for completed tiles
            done = min(col // P, RTILES)
            while built < done:
                t = built
                build_tile(0, t, hT[0][:, t * P:(t + 1) * P])
                built += 1
                if built % GT == 0:
                    fire_group(0, built // GT - 1)

        # ---- conv layers
        nconv = 0 if STAGE <= 1 else (2 if STAGE >= 9 else 1)
        for li in range(nconv):
            if STAGE <= 2:
                continue
            oa = ob = 0
            for t in range(RTILES):
                ka, kb = sched[t]
                T = ka + kb
                T3 = 3 * ((T + 2) // 3)
                G = work.tile([P, T, ROWB], f16, tag="G",
                              bufs=int(os.environ.get("GAT_GBUFS", "3")))
                for g0 in range(0, ka, 8):
                    n = min(8, ka - g0)
                    nc.gpsimd.dma_gather(
                        G[:, g0:g0 + n, :], table_ap[li][0:WINA, :],
                        sb_idxa[:, oa + g0 * 8:oa + (g0 + n) * 8],
                        n * P, n * P, ROWB, queue_num=next_q())
                for g0 in range(0, kb, 8):
                    n = min(8, kb - g0)
                    nc.gpsimd.dma_gather(
                        G[:, ka + g0:ka + g0 + n, :],
                        table_ap[li][WINB_BASE:NROWS, :],
                        sb_idxb[:, ob + g0 * 8:ob + (g0 + n) * 8],
                        n * P, n * P, ROWB, queue_num=next_q())
                oa += ka * 8
                ob += kb * 8
                if STAGE == 3:
                    continue

                alph = work.tile([P, T, H], f16, tag="alph")
                nc.vector.tensor_tensor(out=alph[:],
                                        in0=G[:, :, HID:HID + H],
                                        in1=bc_ap(adst_all[:, li, t, :], T),
                                        op=mybir.AluOpType.add)
                # leaky relu on DVE: max(a, 0.2a)
                nc.vector.scalar_tensor_tensor(out=alph[:], in0=alph[:],
                                               scalar=0.2, in1=alph[:],
                                               op0=mybir.AluOpType.mult,
                                               op1=mybir.AluOpType.max)
                RHS = work.tile([P, T3, HID + H], f16, tag="RHS")
                if T3 > T:
                    nc.vector.memset(RHS[:, T:T3, :], 0)
                nc.scalar.activation(out=RHS[:, 0:T, HID:HID + H], in_=alph[:],
                                     func=mybir.ActivationFunctionType.Exp)
                ex_b = RHS[:, 0:T, HID:HID + H]
                nc.vector.tensor_tensor(
                    out=RHS[:, 0:T, 0:HID].rearrange("p t (h c) -> p t h c", h=H),
                    in0=G[:, :, 0:HID].rearrange("p t (h c) -> p t h c", h=H),
                    in1=app_ap(ex_b, C), op=mybir.AluOpType.mult)

                if STAGE == 36:
                    continue
                ps2 = psC.tile([P, 3, HID + H], f32, tag="cv")
                ng = T3 // 3
                for g in range(ng):
                    nc.tensor.matmul(ps2[:], lhsT=ident16[:],
                                     rhs=RHS[:, 3 * g:3 * g + 3, :],
                                     start=(g == 0), stop=(g == ng - 1))
                nm2 = ep.tile([P, HID + H], f32, tag="nm2")
                ps2v = ps2[:]
                ps2_sw = bass.AP(
                    tensor=ps2v.tensor, offset=ps2v.offset,
                    ap=[list(ps2v.ap[0]), [1, HID + H], [HID + H, 3]])
                nc.vector.reduce_sum(out=nm2[:], in_=ps2_sw,
                                     axis=mybir.AxisListType.X)

                den = ep.tile([P, H], f32, tag="den")
                if t == RTILES - 1:
                    # +eps so empty dst rows (no edges, tail of the last
                    # tile) yield 0/eps = 0, not NaN
                    nc.vector.tensor_scalar_add(out=nm2[:, HID:HID + H],
                                                in0=nm2[:, HID:HID + H],
                                                scalar1=1e-30)
                nc.vector.reciprocal(out=den[:], in_=nm2[:, HID:HID + H])
                hb = ep.tile([P, HID], f32, tag="hb")
                nc.vector.tensor_tensor(
                    out=hb[:].rearrange("p (h c) -> p h c", h=H),
                    in0=nm2[:, 0:HID].rearrange("p (h c) -> p h c", h=H),
                    in1=app_ap(den[:], C), op=mybir.AluOpType.mult)
                if not cb_is0:
                    nc.vector.tensor_tensor(out=hb[:], in0=hb[:], in1=sb_cb[li][:],
                                            op=mybir.AluOpType.add)
                stats = ep.tile([P, nc.vector.BN_STATS_DIM], f32, tag="st")
                nc.vector.bn_stats(out=stats[:], in_=hb[:])
                mv = ep.tile([P, nc.vector.BN_AGGR_DIM], f32, tag="mv")
                nc.vector.bn_aggr(out=mv[:], in_=stats[:])
                sq = ep.tile([P, 1], f32, tag="sq")
                nc.scalar.activation(out=sq[:], in_=mv[:, 1:2],
                                     func=mybir.ActivationFunctionType.Sqrt,
                                     bias=eps_t[:])
                nc.vector.reciprocal(out=sq[:], in_=sq[:])
                if g_is1 and b_is0 and pa_scalar is not None:
                    # fused (hb - mean) * rstd then prelu, on the scalar engine
                    nbias = ep.tile([P, 1], f32, tag="nb")
                    nc.vector.scalar_tensor_tensor(out=nbias[:], in0=mv[:, 0:1],
                                                   scalar=-1.0, in1=sq[:],
                                                   op0=mybir.AluOpType.mult,
                                                   op1=mybir.AluOpType.mult)
                    hb2 = ep.tile([P, HID], f32, tag="hb2")
                    nc.scalar.activation(out=hb2[:], in_=hb[:],
                                         func=mybir.ActivationFunctionType.Prelu,
                                         bias=nbias[:], scale=sq[:],
                                         alpha=float(pa_scalar))
                    hb = hb2
                else:
                    # hb = (hb - mean) * rstd
                    nc.vector.tensor_scalar(out=hb[:], in0=hb[:], scalar1=mv[:, 0:1],
                                            scalar2=sq[:, 0:1],
                                            op0=mybir.AluOpType.subtract,
                                            op1=mybir.AluOpType.mult)
                    if not g_is1:
                        nc.vector.tensor_tensor(out=hb[:], in0=hb[:], in1=sb_g[li][:],
                                                op=mybir.AluOpType.mult)
                    if not b_is0:
                        nc.vector.tensor_tensor(out=hb[:], in0=hb[:], in1=sb_bln[li][:],
                                                op=mybir.AluOpType.add)
                    # prelu
                    t2 = ep.tile([P, HID], f32, tag="t2")
                    if pa_scalar is not None:
                        nc.vector.tensor_scalar(out=t2[:], in0=hb[:], scalar1=0.0,
                                                scalar2=float(pa_scalar),
                                                op0=mybir.AluOpType.min,
                                                op1=mybir.AluOpType.mult)
                    else:
                        nc.vector.tensor_scalar(out=t2[:], in0=hb[:], scalar1=0.0,
                                                scalar2=None, op0=mybir.AluOpType.min)
                        nc.vector.tensor_tensor(out=t2[:], in0=t2[:], in1=sb_pa[li][:],
                                                op=mybir.AluOpType.mult)
                    nc.vector.scalar_tensor_tensor(out=hb[:], in0=hb[:], scalar=0.0,
                                                   in1=t2[:], op0=mybir.AluOpType.max,
                                                   op1=mybir.AluOpType.add)

                if li == 0:
                    pst = psT.tile([P, P], f32, tag="tr")
                    nc.tensor.transpose(out=pst[:], in_=hb[:], identity=identf[:])
                    nc.vector.tensor_copy(out=h1t[t][:], in_=pst[:])
                    # conv2 table build rides the conv1 edge loop so its
                    # group collectives overlap conv1 edge processing
                    build_tile(1, t, h1t[t][:])
                    if (t + 1) % GT == 0:
                        fire_group(1, (t + 1) // GT - 1)
                else:
                    om = ep.tile([P, HID], f32, tag="om")
                    nc.vector.tensor_tensor(out=om[:], in0=hb[:], in1=sb_outw[:],
                                            op=mybir.AluOpType.mult)
                    ov = ep.tile([P, 1], f32, tag="ov")
                    nc.vector.reduce_sum(out=ov[:], in_=om[:],
                                         axis=mybir.AxisListType.X)
                    nc.vector.tensor_scalar_add(out=ov[:], in0=ov[:],
                                                scalar1=sb_outb[:, 0:1])
                    nc.sync.dma_start(out=out[t * P:(t + 1) * P, :], in_=ov[:])

        for p in (psT, psC, psB, psA, ep, work, io, persist, consts):
            p.release()

    nc.compile()
    return nc


# ---------------------------------------------------------------- entry point

def kernel(x, num_x, num_mask, txt_x, txt_mask, edge_index,
           num_proj_w, num_proj_b, txt_proj_w, txt_proj_b,
           node_proj_w, node_proj_b, prelu0_a,
           conv1_w, att_src1, att_dst1, bias1, norm1_g, norm1_b, prelu1_a,
           conv2_w, att_src2, att_dst2, bias2, norm2_g, norm2_b, prelu2_a,
           out_w, out_b, _trace=False):
    x = np.asarray(x, np.float32)
    edge_index = np.asarray(edge_index)

    g_is1 = bool(np.all(norm1_g == 1) and np.all(norm2_g == 1))
    b_is0 = bool(np.all(norm1_b == 0) and np.all(norm2_b == 0))
    cb_is0 = bool(np.all(np.asarray(bias1) == 0) and np.all(np.asarray(bias2) == 0))
    pa1a = np.asarray(prelu1_a, np.float32)
    pa2a = np.asarray(prelu2_a, np.float32)
    pa_scalar = float(pa1a[0]) if (np.all(pa1a == pa1a[0])
                                   and np.all(pa2a == pa1a[0])) else None
    flags = (g_is1, b_is0, cb_is0, pa_scalar)

    pre_key = (hash(edge_index.tobytes()), flags)
    if pre_key in _cache:
        pre, nc = _cache[pre_key]
    else:
        pre = _preprocess(edge_index)
        nc = _build(pre["sched"], flags)
        _cache[pre_key] = (pre, nc)

    numv = (np.asarray(num_x, np.float32)[:, 0] * np.asarray(num_mask, np.float32))
    txtv = np.asarray(txt_x, np.float32) * np.asarray(txt_mask, np.float32)[:, None]
    bias0 = (np.asarray(num_proj_b) + np.asarray(txt_proj_b)
             + np.asarray(node_proj_b)).astype(np.float32)

    padrow = np.zeros((1, ROWB), np.float32)
    padrow[0, HID:HID + H] = ASR_PAD

    shared = {
        "npwT": np.ascontiguousarray(np.asarray(node_proj_w, np.float32).T).astype(F16),
        "tpwT": np.ascontiguousarray(np.asarray(txt_proj_w, np.float32).T).astype(F16),
        "numwT": np.ascontiguousarray(np.asarray(num_proj_w, np.float32).T).astype(F16),
        "bias0": bias0[:, None],
        "prelu0a": np.asarray(prelu0_a, np.float32)[:, None],
        "w1ext": _wext(np.asarray(conv1_w, np.float32),
                       np.asarray(att_src1, np.float32),
                       np.asarray(att_dst1, np.float32)).astype(F16),
        "w2ext": _wext(np.asarray(conv2_w, np.float32),
                       np.asarray(att_src2, np.float32),
                       np.asarray(att_dst2, np.float32)).astype(F16),
        "padrow": padrow.astype(F16),
        "cb1": _bc(bias1), "g1": _bc(norm1_g), "bln1": _bc(norm1_b), "pa1": _bc(prelu1_a),
        "cb2": _bc(bias2), "g2": _bc(norm2_g), "bln2": _bc(norm2_b), "pa2": _bc(prelu2_a),
        "outw": _bc(np.asarray(out_w, np.float32)[0]),
        "outb": np.full((P, 1), np.asarray(out_b, np.float32)[0], np.float32),
    }

    in_maps = []
    for c in range(NCORES):
        nodes = pre["nodes_of_core"][c]
        xTa = np.zeros((EMB, SHARD), np.float32)
        xTa[:, :REAL] = x[nodes].T
        txtTa = np.zeros((TXT, SHARD), np.float32)
        txtTa[:, :REAL] = txtv[nodes].T
        numTa = np.zeros((1, SHARD), np.float32)
        numTa[0, :REAL] = numv[nodes]
        m = dict(shared)
        m["xT"] = xTa.astype(F16)
        m["txtT"] = txtTa.astype(F16)
        m["numT"] = numTa.astype(F16)
        m["idxa"] = pre["idxa"][c]
        m["idxb"] = pre["idxb"][c]
        in_maps.append(m)

    res = run_bass_kernel_spmd(nc, in_maps, core_ids=list(range(NCORES)),
                               trace=_trace)
    out_full = np.zeros(N, np.float32)
    for c in range(NCORES):
        out_full[pre["nodes_of_core"][c]] = res.results[c]["out"][:REAL, 0]
    if _trace:
        kernel._last_exec_ns = res.exec_time_ns
        kernel._last_trace = res.instructions_and_trace
    return out_full
